# revision 18
# baseline (speedup 1.0000x reference)
"""Trainium2 Bass kernel for nn_AttnBlock: LayerNorm -> 16-head attention -> out-proj.

Full inputs in, full output out. Sharding: 8 cores = 2 batches x 4 head-groups
(4 heads per core). Each core computes LN + QKV (its 256 feature slice) +
attention for its 4 heads + a partial output projection; the host sums the 4
partials per batch and adds the output bias.

v6 schedule (evolved from v5):
  - x chunks + wk/wv interleaved across the two hardware DMA queues
    (sync/scalar) so weights land mid-window; wq/wo ride the gpsimd DGE.
  - LN stats accumulate into ONE [2,L] PSUM tile (mean row 0, sq row 1,
    4 banks) so kb0's 4 held banks coexist; kb0 mains interleave with the
    stats matmuls chunk-by-chunk and chase the DMA.
  - stats postprocess runs in PARTITION-PARALLEL [128,16] column space:
    engine row-copies (ScalarE + DVE/gpsimd halves) -> two rearranged
    SBUF->SBUF DMAs -> tiny DVE/ScalarE col chain -> direct r_cols, plus
    two small DMAs to build the xch8 aug rows and the 16 r row-vectors
    (r broadcast via 16 K=1 matmuls).  This replaces ~27us of serial
    single-partition row ops with ~5us.
  - V' computed per head-pair half: h01 halves for tiles 0-7 up front
    (aug+scale deferred to vfix fillers), tiles 8-15 as fused filler units
    (aug accumulated in PSUM, single fused scale drain); all h23 halves are
    relaxed fillers due before the pair-1 pass.
  - attention per (head-pair, 512-q-slab, k-tile): both heads' scores in one
    [128,1024] f32 PSUM tile via concurrent row-group matmuls, ONE exp
    ACTIVATE per k-tile with 1/8 scale, AV lags 3 k-tiles.
  - fillers are fine-grained (<=1.6k cycles), deadline-ordered, and proj
    units are gated until their slab's normalize has had 3 k-tiles to
    finish, so the ScalarE exp stream never starves at slab boundaries.
"""

import os
from contextlib import ExitStack

import numpy as np

import concourse.bass as bass
import concourse.tile as tile
from concourse import bacc, mybir
from concourse.bass_utils import run_bass_kernel_spmd

F32 = mybir.dt.float32
BF16 = mybir.dt.bfloat16

B, L, D = 2, 2048, 1024
NH_TOT, HS = 16, 64
NCORES = 8
HPC = 4                  # heads per core
FPC = HPC * HS           # 256 features per core
P = 128
DCH = D // P             # 8 x^T chunks
KCH = DCH + 1            # +1 augmented chunk
QS = 512                 # q slab
NQS = L // QS            # 4
KT = L // P              # 16 k tiles
TT = L // P              # 16 token tiles
EPS = 1e-5
SCALE = float(HS) ** -0.5
AVLAG = 3

LAST_RESULTS = None


def _build_nc():
    nc = bacc.Bacc("TRN2", target_bir_lowering=False, debug=False)

    xT = nc.dram_tensor("xT", [D, L], BF16, kind="ExternalInput").ap()
    wq = nc.dram_tensor("wq", [KCH * P, FPC], BF16, kind="ExternalInput").ap()
    wk = nc.dram_tensor("wk", [KCH * P, FPC], BF16, kind="ExternalInput").ap()
    wv = nc.dram_tensor("wv", [KCH * P, FPC], BF16, kind="ExternalInput").ap()
    wo = nc.dram_tensor("wo", [FPC, D], BF16, kind="ExternalInput").ap()
    out = nc.dram_tensor("out", [L, D], BF16, kind="ExternalOutput").ap()

    with tile.TileContext(nc) as tc, ExitStack() as ctx:
        persist = ctx.enter_context(tc.tile_pool(name="persist", bufs=1))

        # ---------------- persistent tiles ----------------
        eps_t = persist.tile([P, 1], F32, name="eps")
        nc.vector.memset(eps_t[:], EPS)
        ones_bf = persist.tile([P, 1], BF16, name="ones_bf")
        nc.vector.memset(ones_bf[:], 1.0)
        ones_row = persist.tile([1, P], BF16, name="ones_row")
        nc.vector.memset(ones_row[:], 1.0)
        dummy = persist.tile([P, 1], F32, name="dummy")

        xch = [persist.tile([P, L], BF16, name=f"x{c}") for c in range(DCH)]
        xch8 = persist.tile([P, L], BF16, name="x8")       # augmented rows
        r_bcast = persist.tile([P, L], F32, name="r_bcast")
        qbar = [persist.tile([P, L], BF16, name=f"qb{i}") for i in range(2)]
        kbar = [persist.tile([P, L], BF16, name=f"kb{i}") for i in range(2)]
        vprime = [persist.tile([P, HPC, HS + 2], BF16, name=f"vp{t}")
                  for t in range(TT)]
        onrm = [persist.tile([P, L], BF16, name=f"on{i}") for i in range(2)]
        r_cols = persist.tile([P, TT], F32, name="rcol")
        r_row_bf = persist.tile([1, L], BF16, name="rrow")
        mustd_bf = persist.tile([P, 2, TT], BF16, name="mustd")

        # ---- input DMA: x chunks + wk/wv interleaved on the two hardware
        # queues so weights arrive mid-window; wq/wo on the gpsimd DGE ----
        wp = ctx.enter_context(tc.tile_pool(name="wp", bufs=1))
        wv_t = [wp.tile([P, FPC], BF16, name=f"wv{c}") for c in range(KCH)]
        wk_t = [wp.tile([P, FPC], BF16, name=f"wk{c}") for c in range(KCH)]
        wq_t = [wp.tile([P, FPC], BF16, name=f"wq{c}") for c in range(KCH)]
        wo_t = [wp.tile([P, D], BF16, name=f"wo{ch}") for ch in range(2)]

        def dma_x(eng, c):
            eng.dma_start(out=xch[c][:], in_=xT[P * c:P * (c + 1), :])

        def dma_w(eng, tiles, src, cs):
            for c in cs:
                eng.dma_start(out=tiles[c][:], in_=src[P * c:P * (c + 1), :])

        dma_x(nc.sync, 0)
        dma_x(nc.scalar, 1)
        dma_w(nc.sync, wk_t, wk, range(0, 5))
        dma_w(nc.scalar, wk_t, wk, range(5, KCH))
        dma_x(nc.sync, 2)
        dma_x(nc.scalar, 3)
        dma_x(nc.sync, 4)
        dma_x(nc.scalar, 5)
        dma_w(nc.sync, wv_t, wv, range(0, 5))
        dma_w(nc.scalar, wv_t, wv, range(5, KCH))
        dma_x(nc.sync, 6)
        dma_x(nc.scalar, 7)
        dma_w(nc.sync, wq_t, wq, range(0, 5))
        dma_w(nc.scalar, wq_t, wq, range(5, KCH))
        # wo rides the hardware queues too, but is emitted after the stats
        # bounce DMAs (it is not needed until the out-projection fillers)

        nc.vector.memset(xch8[:], 0.0)
        for t in range(TT):
            nc.vector.memset(vprime[t][:, :, HS:HS + 1], 1.0)
            nc.vector.memset(vprime[t][:, :, HS + 1:HS + 2], 0.0)

        rowstk = ExitStack()
        rowp = rowstk.enter_context(tc.tile_pool(name="rowp", bufs=1))
        mps_sb = rowp.tile([1, L], F32, name="mps_sb")
        sps_sb = rowp.tile([1, L], F32, name="sps_sb")
        musum_c = rowp.tile([P, TT], F32, name="musum_c")
        sqsum_c = rowp.tile([P, TT], F32, name="sqsum_c")
        mu_c = rowp.tile([P, TT], F32, name="mu_c")
        tsq_c = rowp.tile([P, TT], F32, name="tsq_c")
        var_c = rowp.tile([P, TT], F32, name="var_c")
        std_c = rowp.tile([P, TT], F32, name="std_c")
        rscr_c = rowp.tile([P, TT], F32, name="rscr_c")
        r_bf_c = rowp.tile([P, TT], BF16, name="r_bf_c")

        with ExitStack() as bstk:
            scrp = bstk.enter_context(
                tc.tile_pool(name="scrp", bufs=1, space="DRAM"))
            qkps = bstk.enter_context(
                tc.tile_pool(name="qkps", bufs=4, space="PSUM"))
            astk = ExitStack()
            statps = astk.enter_context(
                tc.tile_pool(name="statps", bufs=1, space="PSUM"))
            sqp = astk.enter_context(tc.tile_pool(name="sqp", bufs=2))

            # -------- phase A: LN stats + kb0 mains, paced by chunk DMA ----
            # mean sums -> stat2 row 0, square sums -> stat2 row 1 (one
            # 4-bank PSUM tile); x^2 split between ScalarE and DVE halves.
            stat2 = statps.tile([33, L], F32, name="stat2")
            kb0ps = [qkps.tile([P, 512], F32, name="kb0ps") for _ in range(4)]
            for c in range(DCH):
                sq = sqp.tile([P, L], BF16, name="sqt")
                nc.scalar.activation(
                    out=sq[:, 0:1024], in_=xch[c][:, 0:1024],
                    func=mybir.ActivationFunctionType.Square, scale=1.0)
                nc.vector.tensor_mul(
                    sq[:, 1024:2048], xch[c][:, 1024:2048],
                    xch[c][:, 1024:2048])
                for s in range(L // 512):
                    sl = slice(512 * s, 512 * (s + 1))
                    nc.tensor.matmul(
                        stat2[0:1, sl], ones_bf[:], xch[c][:, sl],
                        start=(c == 0), stop=(c == DCH - 1))
                    nc.tensor.matmul(
                        stat2[32:33, sl], ones_bf[:], sq[:, sl],
                        start=(c == 0), stop=(c == DCH - 1))
                # kb0 mains for this chunk (needs wk; lands mid-window).
                # chunk 0 is deferred to c==1 so the PE isn't blocked on wk
                # before the early stats matmuls, and is emitted FIRST there
                # so the accumulation group's start=True executes first.
                if c >= 1:
                    if c == 1:
                        for s in range(4):
                            sl = slice(512 * s, 512 * (s + 1))
                            nc.tensor.matmul(
                                kb0ps[s][:], wk_t[0][:, 0:P], xch[0][:, sl],
                                start=True, stop=False)
                    for s in range(4):
                        sl = slice(512 * s, 512 * (s + 1))
                        nc.tensor.matmul(
                            kb0ps[s][:], wk_t[c][:, 0:P], xch[c][:, sl],
                            start=False, stop=False)

            # -------- stats postprocess, partition-parallel col space ------
            # drain the two stat rows to SBUF (ScalarE + DVE/gpsimd halves)
            nc.scalar.activation(
                out=mps_sb[:], in_=stat2[0:1, :],
                func=mybir.ActivationFunctionType.Copy, scale=1.0)
            nc.vector.tensor_copy(sps_sb[:], stat2[32:33, :])
            astk.close()

            # transpose rows into [128,16] col space (token = i*128 + p)
            # via a DRAM bounce (SBUF<->SBUF DMAs cannot cross partitions)
            scr_m = scrp.tile([1, L], F32, name="scr_m")
            scr_s = scrp.tile([1, L], F32, name="scr_s")
            nc.sync.dma_start(out=scr_m[0, :], in_=mps_sb[0:1, :])
            nc.scalar.dma_start(out=scr_s[0, :], in_=sps_sb[0:1, :])
            nc.sync.dma_start(
                out=musum_c[:],
                in_=scr_m[0, :].rearrange("(i p) -> p i", p=P))
            nc.scalar.dma_start(
                out=sqsum_c[:],
                in_=scr_s[0, :].rearrange("(i p) -> p i", p=P))

            # col chain: mu, -mu, var, std, r  (all [128,16])
            nc.vector.tensor_scalar_mul(mu_c[:], musum_c[:], 1.0 / D)
            nc.vector.tensor_scalar_mul(
                mustd_bf[:, 0, :], musum_c[:], -1.0 / D)
            nc.vector.tensor_mul(tsq_c[:], mu_c[:], mu_c[:])
            nc.vector.tensor_scalar_mul(var_c[:], sqsum_c[:], 1.0 / D)
            nc.vector.tensor_sub(var_c[:], var_c[:], tsq_c[:])
            nc.scalar.activation(
                out=std_c[:], in_=var_c[:],
                func=mybir.ActivationFunctionType.Sqrt,
                bias=eps_t[:], scale=1.0)
            # preload the exp table set right after the last sqrt use
            nc.scalar.activation(
                out=dummy[:], in_=eps_t[:],
                func=mybir.ActivationFunctionType.Exp, scale=1.0)
            nc.vector.tensor_copy(mustd_bf[:, 1, :], std_c[:])
            nc.vector.reciprocal_approx_accurate(
                out=r_cols[:], in_=std_c[:], scratch=rscr_c[:])
            nc.vector.tensor_copy(r_bf_c[:], r_cols[:])

            # aug rows (-mu, std) -> xch8 rows 0/1; r -> row vector
            # (again via DRAM: linear (p, r, i) out, strided re-read)
            scr_ms = scrp.tile([1, P * 2 * TT], BF16, name="scr_ms")
            scr_r = scrp.tile([1, P * TT], BF16, name="scr_r")
            nc.scalar.dma_start(
                out=scr_ms[0, :].rearrange("(p x) -> p x", p=P),
                in_=mustd_bf[:, :, :])
            nc.sync.dma_start(
                out=scr_r[0, :].rearrange("(p i) -> p i", p=P),
                in_=r_bf_c[:, :])
            nc.scalar.dma_start(
                out=xch8[0:2, :].rearrange("r (i p) -> r i p", p=P),
                in_=scr_ms[0, :].rearrange("(p r i) -> r i p", p=P, r=2))
            nc.sync.dma_start(
                out=r_row_bf[0:1, :].rearrange("r (i p) -> r i p", p=P),
                in_=scr_r[0, :].rearrange("(p i) -> i p", p=P))

            for ch in range(2):
                eng = nc.sync if ch == 0 else nc.scalar
                eng.dma_start(out=wo_t[ch][:], in_=wo[P * ch:P * (ch + 1), :])

            # vpp opens only after the stats PSUM tile is released (8-bank
            # budget: stat2 4 + kb0ps 4 during phase A)
            vpp = bstk.enter_context(
                tc.tile_pool(name="vpp", bufs=2, space="PSUM"))

            # -------- V' h01 halves for tiles 0-7 (aug/scale deferred) -----
            def vh01_group(t0):
                pv4 = vpp.tile([P, 512], F32, name="pv4")
                for t in range(t0, t0 + 4):
                    pv = pv4[:, P * (t - t0):P * (t - t0 + 1)]
                    for c in range(DCH):
                        nc.tensor.matmul(
                            pv, xch[c][:, P * t:P * (t + 1)],
                            wv_t[c][:, 0:P],
                            start=(c == 0 and t == t0),
                            stop=(c == DCH - 1 and t == t0 + 3))
                for t in range(t0, t0 + 4):
                    pv = pv4[:, P * (t - t0):P * (t - t0 + 1)]
                    nc.vector.tensor_copy(
                        vprime[t][:, 0:2, 0:HS],
                        pv.rearrange("p (h f) -> p h f", h=2))

            vh01_group(0)
            vh01_group(4)

            # -------- kb0 augs (PE) ----
            for s in range(4):
                sl = slice(512 * s, 512 * (s + 1))
                nc.tensor.matmul(
                    kb0ps[s][:], wk_t[DCH][:, 0:P], xch8[:, sl],
                    start=False, stop=True)

            # r broadcast: K=1 matmuls from the r row vector
            for s in range(4):
                sl = slice(512 * s, 512 * (s + 1))
                rb = vpp.tile([P, 512], F32, name="pv4")
                nc.tensor.matmul(
                    rb[:], ones_row[:], r_row_bf[0:1, sl],
                    start=True, stop=True)
                nc.vector.tensor_copy(r_bcast[:, sl], rb[:])

            # kb0 scaled drains (DVE) — emitted after the r_bcast copies so
            # the in-order DVE queue never waits on a later instruction
            for s in range(4):
                sl = slice(512 * s, 512 * (s + 1))
                nc.vector.tensor_mul(
                    kbar[0][:, sl], kb0ps[s][:], r_bcast[:, sl])

            # ---- qbar[0] slab 0 ----
            qps = qkps.tile([P, 512], F32, name="kb0ps")
            for c in range(KCH):
                rhs = xch[c] if c < DCH else xch8
                nc.tensor.matmul(
                    qps[:], wq_t[c][:, 0:P], rhs[:, 0:512],
                    start=(c == 0), stop=(c == KCH - 1))
            nc.vector.tensor_mul(
                qbar[0][:, 0:512], qps[:], r_bcast[:, 0:512])

        rowstk.close()

        # ----------------- filler work-list for attention phase -----------
        # fine-grained matmul groups issued inside the attention loop to fill
        # the PE slack under the exp-bound pacing; consumed strictly in order
        # with an optional not-before global-kt gate.
        filler_units = []

        def vfix01_unit(t0):
            # deferred aug + (raw+aug)*r for 4 h01 half-tiles
            def go(aux_pool):
                va = aux_pool.tile([P, 512], F32, name="aux")
                for j, t in enumerate(range(t0, t0 + 4)):
                    nc.tensor.matmul(
                        va[:, P * j:P * (j + 1)],
                        xch8[:, P * t:P * (t + 1)], wv_t[DCH][:, 0:P],
                        start=(j == 0), stop=(j == 3))
                for j, t in enumerate(range(t0, t0 + 4)):
                    nc.vector.tensor_add(
                        vprime[t][:, 0:2, 0:HS], vprime[t][:, 0:2, 0:HS],
                        va[:, P * j:P * (j + 1)].rearrange(
                            "p (h f) -> p h f", h=2))
                    nc.vector.tensor_scalar_mul(
                        vprime[t][:, 0:2, 0:HS], vprime[t][:, 0:2, 0:HS],
                        r_cols[:, t:t + 1])
            return go

        def vhalf_unit(t, hb):
            # complete V' half tile: mains + aug in PSUM, fused scale drain
            def go(aux_pool):
                pv = aux_pool.tile([P, 512], F32, name="aux")
                for c in range(KCH):
                    lhs = xch[c] if c < DCH else xch8
                    nc.tensor.matmul(
                        pv[:, 0:P], lhs[:, P * t:P * (t + 1)],
                        wv_t[c][:, hb:hb + P],
                        start=(c == 0), stop=(c == KCH - 1))
                nc.vector.tensor_scalar_mul(
                    vprime[t][:, 2 * (hb // P):2 * (hb // P) + 2, 0:HS],
                    pv[:, 0:P].rearrange("p (h f) -> p h f", h=2),
                    r_cols[:, t:t + 1])
            return go

        def qk_unit(wt, dst, m, s):
            # one 512-token slab of a q/k projection, split in four pumps
            cell = {}

            def mk(c0, c1, first, last):
                def go(aux_pool):
                    sl = slice(512 * s, 512 * (s + 1))
                    if first:
                        cell["pq"] = aux_pool.tile([P, 512], F32, name="aux")
                    pq = cell["pq"]
                    for c in range(c0, c1):
                        rhs = xch[c] if c < DCH else xch8
                        nc.tensor.matmul(
                            pq[:], wt[c][:, P * m:P * (m + 1)], rhs[:, sl],
                            start=(c == 0), stop=(c == KCH - 1))
                    if last:
                        nc.vector.tensor_mul(
                            dst[m][:, sl], pq[:], r_bcast[:, sl])
                return go
            return [(1536, mk(0, 3, True, False)),
                    (1024, mk(3, 5, False, False)),
                    (1024, mk(5, 7, False, False)),
                    (1224, mk(7, KCH, False, True))]

        # deadline-ordered filler list (consumed strictly in order).
        # entries: (cost, go, min_ktg, due) -- `due` is the global k-tile
        # iteration at whose start the unit's output is first read; pump()
        # force-runs overdue units regardless of credit, which makes the
        # emission order (= dependency order) correct by construction.
        NEVER = 10 ** 9
        filler_units.append((700, vfix01_unit(0), 0, 3))
        filler_units.append((700, vfix01_unit(4), 0, 7))
        for t in range(8, TT):
            filler_units.append((1280, vhalf_unit(t, 0), 0, min(t + 3, 15)))
        for s in range(1, 4):
            for cost, go in qk_unit(wq_t, qbar, 0, s):
                filler_units.append((cost, go, 0, 16 * s))
        filler_units.append((1280, vhalf_unit(0, P), 0, 67))
        for s in range(4):
            for cost, go in qk_unit(wk_t, kbar, 1, s):
                filler_units.append((cost, go, 0, 64 + 4 * s))
        for cost, go in qk_unit(wq_t, qbar, 1, 0):
            filler_units.append((cost, go, 0, 64))
        for t in range(1, TT):
            filler_units.append(
                (1280, vhalf_unit(t, P), 0, min(64 + t + 3, 79)))
        for s in range(1, 4):
            for cost, go in qk_unit(wq_t, qbar, 1, s):
                filler_units.append((cost, go, 0, 64 + 16 * s))

        def proj_unit(t, s2, use_scalar, late):
            def go(aux_pool):
                po = aux_pool.tile([P, 512], F32, name="aux")
                for ch in range(2):
                    nc.tensor.matmul(
                        po[:], onrm[ch][:, P * t:P * (t + 1)],
                        wo_t[ch][:, 512 * s2:512 * (s2 + 1)],
                        start=(ch == 0), stop=(ch == 1))
                ot = ostg_pool.tile([P, 512], BF16, name="ot")
                if use_scalar:
                    nc.scalar.copy(ot[:], po[:])
                else:
                    nc.vector.tensor_copy(ot[:], po[:])
                # late slabs ride the (idle) sync hardware ring so the final
                # dbc broadcasts aren't stuck behind gpsimd software copies
                eng = nc.sync if late else nc.gpsimd
                eng.dma_start(
                    out=out[P * t:P * (t + 1), 512 * s2:512 * (s2 + 1)],
                    in_=ot[:])
            return go

        with ExitStack() as cstk:
            spool = cstk.enter_context(
                tc.tile_pool(name="spool", bufs=2, space="PSUM"))
            opjp = cstk.enter_context(
                tc.tile_pool(name="opjp", bufs=1, space="PSUM"))
            auxp = cstk.enter_context(
                tc.tile_pool(name="auxp", bufs=2, space="PSUM"))
            epool = cstk.enter_context(tc.tile_pool(name="epool", bufs=8))
            ostg_pool = cstk.enter_context(tc.tile_pool(name="ostg", bufs=3))
            nrmp = cstk.enter_context(tc.tile_pool(name="nrmp", bufs=2))

            fill_i = 0
            fill_credit = 0
            ktg = 0     # global k-tile counter across all (pair, qs)

            def pump(budget_cycles):
                nonlocal fill_i, fill_credit
                fill_credit = min(fill_credit + budget_cycles, 6000)
                # dependency order == list order, so to satisfy the furthest
                # overdue unit everything before it must run as well
                force_to = fill_i
                for idx in range(fill_i, len(filler_units)):
                    if filler_units[idx][3] <= ktg:
                        force_to = idx + 1
                while fill_i < force_to:
                    cost, go, _mk, _due = filler_units[fill_i]
                    go(auxp)
                    fill_i += 1
                    fill_credit = max(0, fill_credit - cost)
                while fill_i < len(filler_units):
                    cost, go, min_ktg, due = filler_units[fill_i]
                    if ktg < min_ktg or cost > fill_credit:
                        break
                    go(auxp)
                    fill_i += 1
                    fill_credit -= cost

            # ---------------- phase C: attention ----------------
            # per k-tile: both heads' scores into one [128,1024] psum tile
            # (2 banks) via concurrent row groups, ONE exp ACTIVATE straight
            # from PSUM into bf16 SBUF, AV lags AVLAG k-tiles so ScalarE
            # paces and PE never blocks.
            for pair in range(2):
                qb, kb = qbar[pair], kbar[pair]
                for qs in range(NQS):
                    qsl = slice(QS * qs, QS * (qs + 1))
                    ops = [opjp.tile([HS + 2, QS], F32, name=f"op{ho}")
                           for ho in range(2)]
                    e_of = {}

                    def issue_av(kt):
                        for ho in range(2):
                            h = 2 * pair + ho
                            nc.tensor.matmul(
                                ops[ho][:],
                                vprime[kt][:, h, :],
                                e_of[kt][:, 512 * ho:512 * (ho + 1)],
                                start=(kt == 0),
                                stop=(kt == KT - 1))
                        del e_of[kt]

                    for kt in range(KT):
                        pump(0)     # force-run any due units before readers
                        ksl = slice(P * kt, P * (kt + 1))
                        sp = spool.tile([P, 2 * QS], F32, name="sp")
                        for ho in range(2):
                            hb = HS * ho
                            nc.tensor.matmul(
                                sp[:, 512 * ho:512 * (ho + 1)],
                                kb[hb:hb + HS, ksl], qb[hb:hb + HS, qsl],
                                start=True, stop=True)
                        pump(900 if pair == 0 else 980)
                        if kt >= AVLAG:
                            issue_av(kt - AVLAG)
                        e = epool.tile([P, 2 * QS], BF16, name="e")
                        nc.scalar.activation(
                            out=e[:], in_=sp[:],
                            func=mybir.ActivationFunctionType.Exp,
                            scale=SCALE)
                        e_of[kt] = e
                        ktg += 1
                    for kt in range(KT - AVLAG, KT):
                        issue_av(kt)

                    # softmax denominators -> normalize into onrm
                    last_slab = (pair == 1 and qs == NQS - 1)
                    for ho in range(2):
                        den0 = nrmp.tile([1, QS], F32, name="den0")
                        nc.vector.tensor_copy(
                            den0[:], ops[ho][HS:HS + 1, :])
                        oraw = nrmp.tile([HS, QS], F32, name="oraw")
                        nc.vector.tensor_copy(oraw[:], ops[ho][0:HS, :])
                        dscr = nrmp.tile([1, QS], F32, name="dscr")
                        dinv = nrmp.tile([1, QS], F32, name="dinv")
                        nc.vector.reciprocal_approx_accurate(
                            out=dinv[:], in_=den0[:], scratch=dscr[:])
                        dbc = nrmp.tile([HS, QS], F32, name="dbc")
                        nc.gpsimd.partition_broadcast(dbc[:], dinv[:])
                        nc.vector.tensor_mul(
                            onrm[pair][HS * ho:HS * ho + HS, qsl],
                            oraw[:], dbc[:])

                    # out-projection for finished q-slabs becomes filler,
                    # gated 3 k-tiles into the next slab so its normalize
                    # never blocks the score stream
                    if pair == 1:
                        gate = ktg + 3 if not last_slab else 0
                        for t in range(4 * qs, 4 * (qs + 1)):
                            for s2 in range(2):
                                filler_units.append(
                                    (1100,
                                     proj_unit(t, s2, qs == 3, qs >= 2),
                                     gate, NEVER))
                        pump(600)

            # drain any remaining filler (tail of the out-projection)
            while fill_i < len(filler_units):
                filler_units[fill_i][1](auxp)
                fill_i += 1

    nc.compile()
    return nc


_NC = None


def _host_weights(W, bias, ln_g, ln_b, rows):
    Wt = W * ln_g[None, :]
    c = W @ ln_b + bias
    s = Wt.sum(axis=1)
    What = np.zeros((KCH * P, FPC), np.float32)
    What[0:D, :] = Wt[rows].T
    What[D, :] = s[rows]
    What[D + 1, :] = c[rows]
    return What


def kernel(x, ln_g, ln_b, Wq, bq, Wk, bk, Wv, bv, Wo, bo):
    global _NC, LAST_RESULTS
    x = np.ascontiguousarray(np.asarray(x, np.float32))
    ln_g = np.asarray(ln_g, np.float32)
    ln_b = np.asarray(ln_b, np.float32)
    Wq, bq = np.asarray(Wq, np.float32), np.asarray(bq, np.float32)
    Wk, bk = np.asarray(Wk, np.float32), np.asarray(bk, np.float32)
    Wv, bv = np.asarray(Wv, np.float32), np.asarray(bv, np.float32)
    Wo, bo = np.asarray(Wo, np.float32), np.asarray(bo, np.float32)

    if _NC is None:
        _NC = _build_nc()

    import ml_dtypes
    bf = ml_dtypes.bfloat16
    in_maps = []
    for core in range(NCORES):
        b, g = core // HPC, core % HPC
        rows = slice(FPC * g, FPC * (g + 1))
        in_maps.append({
            "xT": np.ascontiguousarray(x[b].T).astype(bf),
            "wq": _host_weights(Wq, bq, ln_g, ln_b, rows).astype(bf),
            "wk": _host_weights(Wk, bk, ln_g, ln_b, rows).astype(bf),
            "wv": _host_weights(Wv, bv, ln_g, ln_b, rows).astype(bf),
            "wo": np.ascontiguousarray(Wo[:, rows].T).astype(bf),
        })

    res = run_bass_kernel_spmd(
        _NC, in_maps, core_ids=list(range(NCORES)),
        trace=bool(int(os.environ.get("KERNEL_TRACE", "0"))),
    )
    LAST_RESULTS = res

    out = np.zeros((B, L, D), np.float32)
    for b in range(B):
        acc = res.results[HPC * b]["out"].astype(np.float32).copy()
        for g in range(1, HPC):
            acc += res.results[HPC * b + g]["out"]
        out[b] = acc + bo[None, :]
    return out
